# revision 50
# baseline (speedup 1.0000x reference)
"""Trainium2 Bass kernel for nn_DummyMoELayer (top-1 MoE, E=8, H=2048, I=128).

Strategy (expert-parallel, per the sharding hint): the host computes the tiny
router (logits -> softmax -> top-1) and dispatches tokens to the 8 NeuronCores
by expert id. Core e receives expert e's tokens (gathered, pre-transposed to
[H, C]) plus that expert's weights (pre-transposed), and computes

    y = c[t] * (silu(x @ wgT) * (x @ wuT)) @ wdT

entirely on device. The host scatters the per-expert outputs back into the
full [T, H] tensor. Top-1 masks are disjoint so the scatter is exact.

Device layouts (all chosen so every DMA is contiguous and no on-device
transposes are needed):
  xt  [KT, 128, C]   x gathered+transposed, K-tile major (KT = H/128)
  wg  [128, KT, I]   w_gate[e].T, partition-major (H on partitions)
  wu  [128, KT, I]   w_up[e].T, partition-major
  wd  [I, H]         w_down[e].T  (I on partitions; single K-tile for down)
  cf  [128, C/128]   top-1 prob per token, partition-major per 128-token block
  y   [C, H]         output rows (OUT_DT; host upcasts + scatters)

C is the per-expert token capacity (max expert load rounded up to 128);
every core runs the same program (SPMD), shorter experts are padded with
token 0 and coefficient 0, and the host scatter only takes real rows.
"""
import numpy as np

import concourse.mybir as mybir
from concourse.bacc import Bacc
from concourse.tile import TileContext
from concourse.bass_utils import run_bass_kernel_spmd
from ml_dtypes import bfloat16

E = 8
H = 2048
I = 128
KT = H // 128
NCORES = 8

# Compute dtype for the expert matmuls: "bf16" (half traffic, ~4e-3 rel err)
# or "f32r" (fp32 inputs at bf16 PE speed for N>=256, ~3e-4 rel err).
COMPUTE_DT = "bf16"
# Output dtype for y: "f32" (exact store) or "bf16" (half write traffic).
OUT_DT = "bf16"

_DT = {
    "bf16": (mybir.dt.bfloat16, bfloat16),
    "f32r": (mybir.dt.float32r, np.float32),
    "f32": (mybir.dt.float32, np.float32),
}

# jax.random.uniform(jax.random.key(1), (8,)) * 0.5 - 0.25  (bit-exact)
_COS_SIM = np.array(
    [1021643840, 3180165600, 1047778164, 1035999368,
     3191439084, 3195509684, 3185013440, 1043121252],
    dtype=np.uint32,
).view(np.float32)

_BUILD_CACHE = {}
LAST_RESULTS = None  # BassKernelResults of the most recent device run


def _ensure_axon_hooks():
    """The agent image's ``antenv`` lacks ``axon_hooks``; under BASS_TRACE=1
    concourse imports it unconditionally. Install a functional stand-in (with
    the real ctypes NTFF hook when the injected .so supports it)."""
    import importlib, sys, types
    try:
        importlib.import_module("antenv.axon_hooks")
        return
    except ImportError:
        pass
    mod = types.ModuleType("antenv.axon_hooks")
    mod._hook = None

    def set_axon_ntff_profile_hook(h):
        mod._hook = h

    def get_axon_ntff_profile_hook():
        return mod._hook

    mod.set_axon_ntff_profile_hook = set_axon_ntff_profile_hook
    mod.get_axon_ntff_profile_hook = get_axon_ntff_profile_hook
    try:
        from trn_agent_boot.trn_boot import _ntff_profile_via_ctypes
        mod._hook = _ntff_profile_via_ctypes("/opt/axon/libaxon_pjrt.so")
    except Exception:
        pass
    sys.modules["antenv.axon_hooks"] = mod
    try:
        import antenv
        antenv.axon_hooks = mod
    except Exception:
        pass


_ensure_axon_hooks()


def _token_tiles(C):
    """Split C tokens into tiles of 512 plus one remainder tile (mult of 128)."""
    sizes = [512] * (C // 512)
    if C % 512:
        sizes.append(C % 512)
    return sizes


def _build(C, dtname, outdtname):
    key = (C, dtname, outdtname)
    if key in _BUILD_CACHE:
        return _BUILD_CACHE[key]
    dt, _ = _DT[dtname]
    odt, _ = _DT[outdtname] if outdtname != "f32" else (mybir.dt.float32, np.float32)
    f32 = mybir.dt.float32
    AF = mybir.ActivationFunctionType

    nc = Bacc("TRN2")
    # wg/wu are shipped partition-major ([128, KT, I]) so the load is one
    # fully contiguous DMA per tensor.
    xt = nc.dram_tensor("xt", [KT, 128, C], dt, kind="ExternalInput")
    wg = nc.dram_tensor("wg", [128, KT, I], dt, kind="ExternalInput")
    wu = nc.dram_tensor("wu", [128, KT, I], dt, kind="ExternalInput")
    wd = nc.dram_tensor("wd", [I, H], dt, kind="ExternalInput")
    cf = nc.dram_tensor("cf", [128, C // 128], f32, kind="ExternalInput")
    y = nc.dram_tensor("y", [C, H], odt, kind="ExternalOutput")

    with TileContext(nc) as tc:
        with (
            tc.tile_pool(name="wpool", bufs=1) as wpool,
            tc.tile_pool(name="xpool", bufs=4) as xpool,
            tc.tile_pool(name="apool", bufs=3) as apool,
            tc.tile_pool(name="ypool", bufs=10) as ypool,
            tc.tile_pool(name="psum_g", bufs=1, space="PSUM") as psum_g,
            tc.tile_pool(name="psum_u", bufs=1, space="PSUM") as psum_u,
            tc.tile_pool(name="psum_y", bufs=6, space="PSUM") as psum_y,
        ):
            wg_s = wpool.tile([128, KT, I], dt, tag="wg")
            wu_s = wpool.tile([128, KT, I], dt, tag="wu")
            wd_s = wpool.tile([128, H], dt, tag="wd")
            cf_s = wpool.tile([128, C // 128], f32, tag="cf")
            # Startup order on the sync ring: wg, then the FIRST x tile, then
            # wu/wd/cf — the first gate matmul only needs wg + one x quarter,
            # and wu/wd are not needed until the up/down phases ~4-10us later.
            tiles = _token_tiles(C)
            xt0_s = xpool.tile([128, KT, 512], dt, tag="x", name="xt0_s")[:, :, :tiles[0]]
            # interleave wg quarters (128KB, contiguous 1KB/partition runs in
            # the partition-major layout) with the first x tile's quarters so
            # the k=0 gate matmul starts after ~0.4MB of DMA
            for q0 in range(0, KT, 4):
                nc.sync.dma_start(wg_s[:, q0:q0 + 4, :], wg[:, q0:q0 + 4, :])
                nc.sync.dma_start(
                    xt0_s[:, q0:q0 + 4, :],
                    xt[q0:q0 + 4, :, 0:tiles[0]].rearrange("k p t -> p k t"))
            nc.sync.dma_start(wu_s[:], wu[:])
            nc.sync.dma_start(wd_s[:], wd[:])
            nc.sync.dma_start(cf_s[:], cf[:])

            def emit_down(a_s, TT, t0):
                # down projection per 128-token subtile; scale by c[t] on the
                # PSUM->SBUF copy (ACT/DVE alternate per 512-col chunk);
                # store rows to HBM
                for j in range(TT // 128):
                    jj = t0 // 128 + j
                    y_s = ypool.tile([128, H], odt, tag="ys", name="y_s")
                    for ci, n0 in enumerate(range(0, H, 512)):
                        y_ps = psum_y.tile([128, 512], f32, tag="yp", name="y_ps")
                        nc.tensor.matmul(y_ps, lhsT=a_s[:, j * 128:(j + 1) * 128],
                                         rhs=wd_s[:, n0:n0 + 512],
                                         start=True, stop=True)
                        if ci % 2 == 0:
                            nc.scalar.activation(y_s[:, n0:n0 + 512], y_ps,
                                                 AF.Copy, scale=cf_s[:, jj:jj + 1])
                        else:
                            nc.vector.tensor_scalar_mul(y_s[:, n0:n0 + 512], y_ps,
                                                        cf_s[:, jj:jj + 1])
                    # stores alternate between the two HWDGE rings
                    eng = nc.scalar if jj % 2 == 0 else nc.sync
                    eng.dma_start(y[t0 + j * 128: t0 + (j + 1) * 128, :], y_s[:])

            t0 = 0
            prev = None
            for ti, TT in enumerate(tiles):
                if ti == 0:
                    xt_s = xt0_s
                else:
                    xt_s = xpool.tile([128, KT, 512], dt, tag="x",
                                      name="xt_s")[:, :, :TT]
                    # quarter-granularity loads so gate matmuls start after
                    # the first 4 k-slices land instead of the whole tile
                    for q0 in range(0, KT, 4):
                        nc.sync.dma_start(
                            xt_s[:, q0:q0 + 4, :],
                            xt[q0:q0 + 4, :, t0:t0 + TT].rearrange("k p t -> p k t"))

                # gate / up projections: gT/uT [I, TT] accumulated over KT
                g_ps = psum_g.tile([128, 512], f32, tag="g", name="g_ps")[:, :TT]
                u_ps = psum_u.tile([128, 512], f32, tag="u", name="u_ps")[:, :TT]
                for k in range(KT):
                    nc.tensor.matmul(g_ps, lhsT=wg_s[:, k, :], rhs=xt_s[:, k, :],
                                     start=(k == 0), stop=(k == KT - 1))
                for k in range(KT):
                    nc.tensor.matmul(u_ps, lhsT=wu_s[:, k, :], rhs=xt_s[:, k, :],
                                     start=(k == 0), stop=(k == KT - 1))

                # a = silu(g) * u   [I, TT] in compute dtype
                sg_s = apool.tile([128, 512], f32, tag="sg", name="sg_s")[:, :TT]
                a_s = apool.tile([128, 512], dt, tag="a", name="a_s")[:, :TT]
                nc.scalar.activation(sg_s, g_ps, AF.Silu)
                nc.vector.tensor_mul(a_s, sg_s, u_ps)

                # software pipeline: emit the PREVIOUS tile's down phase now,
                # so this tile's gate/up matmuls run on the PE while the
                # previous tile's PSUM->SBUF copies drain
                if prev is not None:
                    emit_down(*prev)
                prev = (a_s, TT, t0)
                t0 += TT
            emit_down(*prev)

    nc.finalize()
    _BUILD_CACHE[key] = nc
    return nc


def kernel(hidden_states, gate_w, w_gate, w_up, w_down):
    global LAST_RESULTS
    hidden_states = np.asarray(hidden_states, dtype=np.float32)
    gate_w = np.asarray(gate_w, dtype=np.float32)
    w_gate = np.asarray(w_gate, dtype=np.float32)
    w_up = np.asarray(w_up, dtype=np.float32)
    w_down = np.asarray(w_down, dtype=np.float32)

    B, S, _ = hidden_states.shape
    T = B * S
    x = np.ascontiguousarray(hidden_states.reshape(T, H))

    # --- host router: logits -> softmax -> top-1 (this is the sharding step) ---
    logits = x @ gate_w.T                                   # [T, E] fp32
    m = logits.max(axis=1, keepdims=True)
    ex = np.exp(logits - m)
    probs = ex / ex.sum(axis=1, keepdims=True)              # [T, E] fp32
    sel = np.argmax(logits, axis=1)                         # ties -> first, as jnp
    cvals = probs[np.arange(T), sel].astype(np.float32)     # top-1 prob per token

    idx = [np.nonzero(sel == e)[0] for e in range(E)]
    maxc = max(len(i) for i in idx)
    C = max(128, ((maxc + 127) // 128) * 128)

    dt, npdt = _DT[COMPUTE_DT]
    in_maps = []
    for e in range(E):
        ie = idx[e]
        pad = C - len(ie)
        ie_p = np.concatenate([ie, np.zeros(pad, dtype=ie.dtype)]) if pad else ie
        c_p = cvals[ie_p].copy()
        if pad:
            c_p[len(ie):] = 0.0
        xte = np.ascontiguousarray(x[ie_p].T).reshape(KT, 128, C)
        in_maps.append({
            "xt": xte.astype(npdt, copy=False) if npdt is np.float32 else xte.astype(npdt),
            "wg": np.ascontiguousarray(
                w_gate[e].T.reshape(KT, 128, I).transpose(1, 0, 2)).astype(npdt),
            "wu": np.ascontiguousarray(
                w_up[e].T.reshape(KT, 128, I).transpose(1, 0, 2)).astype(npdt),
            "wd": np.ascontiguousarray(w_down[e].T).astype(npdt),
            "cf": np.ascontiguousarray(c_p.reshape(C // 128, 128).T),
        })

    nc = _build(C, COMPUTE_DT, OUT_DT)
    res = run_bass_kernel_spmd(nc, in_maps, core_ids=list(range(NCORES)))
    LAST_RESULTS = res

    final = np.empty((T, H), dtype=np.float32)
    for e in range(E):
        ie = idx[e]
        if len(ie):
            final[ie] = res.results[e]["y"][:len(ie)].astype(np.float32, copy=False)

    hn = np.zeros((1, B, E * 4), dtype=np.float32)
    return (
        final.reshape(B, S, H),
        probs,
        hn,
        np.float32(0.035),
        _COS_SIM.copy(),
        np.float32(0.019),
    )


# revision 51
# speedup vs baseline: 1.0505x; 1.0505x over previous
"""Trainium2 Bass kernel for nn_DummyMoELayer (top-1 MoE, E=8, H=2048, I=128).

Strategy (expert-parallel, per the sharding hint): the host computes the tiny
router (logits -> softmax -> top-1) and dispatches tokens to the 8 NeuronCores
by expert id. Core e receives expert e's tokens (gathered, pre-transposed to
[H, C]) plus that expert's weights (pre-transposed), and computes

    y = c[t] * (silu(x @ wgT) * (x @ wuT)) @ wdT

entirely on device. The host scatters the per-expert outputs back into the
full [T, H] tensor. Top-1 masks are disjoint so the scatter is exact.

Device layouts (all chosen so every DMA is contiguous and no on-device
transposes are needed):
  xt  [KT, 128, C]   x gathered+transposed, K-tile major (KT = H/128)
  wg  [128, KT, I]   w_gate[e].T, partition-major (H on partitions)
  wu  [128, KT, I]   w_up[e].T, partition-major
  wd  [I, H]         w_down[e].T  (I on partitions; single K-tile for down)
  cf  [128, C/128]   top-1 prob per token, partition-major per 128-token block
  y   [C, H]         output rows (OUT_DT; host upcasts + scatters)

C is the per-expert token capacity (max expert load rounded up to 128);
every core runs the same program (SPMD), shorter experts are padded with
token 0 and coefficient 0, and the host scatter only takes real rows.
"""
import numpy as np

import concourse.mybir as mybir
from concourse.bacc import Bacc
from concourse.tile import TileContext
from concourse.bass_utils import run_bass_kernel_spmd
from ml_dtypes import bfloat16

E = 8
H = 2048
I = 128
KT = H // 128
NCORES = 8

# Compute dtype for the expert matmuls: "bf16" (half traffic, ~4e-3 rel err)
# or "f32r" (fp32 inputs at bf16 PE speed for N>=256, ~3e-4 rel err).
COMPUTE_DT = "bf16"
# Output dtype for y: "f32" (exact store) or "bf16" (half write traffic).
OUT_DT = "bf16"

_DT = {
    "bf16": (mybir.dt.bfloat16, bfloat16),
    "f32r": (mybir.dt.float32r, np.float32),
    "f32": (mybir.dt.float32, np.float32),
}

# jax.random.uniform(jax.random.key(1), (8,)) * 0.5 - 0.25  (bit-exact)
_COS_SIM = np.array(
    [1021643840, 3180165600, 1047778164, 1035999368,
     3191439084, 3195509684, 3185013440, 1043121252],
    dtype=np.uint32,
).view(np.float32)

_BUILD_CACHE = {}
LAST_RESULTS = None  # BassKernelResults of the most recent device run


def _ensure_axon_hooks():
    """The agent image's ``antenv`` lacks ``axon_hooks``; under BASS_TRACE=1
    concourse imports it unconditionally. Install a functional stand-in (with
    the real ctypes NTFF hook when the injected .so supports it)."""
    import importlib, sys, types
    try:
        importlib.import_module("antenv.axon_hooks")
        return
    except ImportError:
        pass
    mod = types.ModuleType("antenv.axon_hooks")
    mod._hook = None

    def set_axon_ntff_profile_hook(h):
        mod._hook = h

    def get_axon_ntff_profile_hook():
        return mod._hook

    mod.set_axon_ntff_profile_hook = set_axon_ntff_profile_hook
    mod.get_axon_ntff_profile_hook = get_axon_ntff_profile_hook
    try:
        from trn_agent_boot.trn_boot import _ntff_profile_via_ctypes
        mod._hook = _ntff_profile_via_ctypes("/opt/axon/libaxon_pjrt.so")
    except Exception:
        pass
    sys.modules["antenv.axon_hooks"] = mod
    try:
        import antenv
        antenv.axon_hooks = mod
    except Exception:
        pass


_ensure_axon_hooks()


def _token_tiles(C):
    """Split C tokens into tiles of 512 plus one remainder tile (mult of 128)."""
    sizes = [512] * (C // 512)
    if C % 512:
        sizes.append(C % 512)
    return sizes


def _build(C, dtname, outdtname):
    key = (C, dtname, outdtname)
    if key in _BUILD_CACHE:
        return _BUILD_CACHE[key]
    dt, _ = _DT[dtname]
    odt, _ = _DT[outdtname] if outdtname != "f32" else (mybir.dt.float32, np.float32)
    f32 = mybir.dt.float32
    AF = mybir.ActivationFunctionType

    nc = Bacc("TRN2")
    # wg/wu are shipped partition-major ([128, KT, I]) so the load is one
    # fully contiguous DMA per tensor.
    xt = nc.dram_tensor("xt", [KT, 128, C], dt, kind="ExternalInput")
    wg = nc.dram_tensor("wg", [128, KT, I], dt, kind="ExternalInput")
    wu = nc.dram_tensor("wu", [128, KT, I], dt, kind="ExternalInput")
    wd = nc.dram_tensor("wd", [I, H], dt, kind="ExternalInput")
    cf = nc.dram_tensor("cf", [128, C // 128], f32, kind="ExternalInput")
    y = nc.dram_tensor("y", [C, H], odt, kind="ExternalOutput")

    with TileContext(nc) as tc:
        with (
            tc.tile_pool(name="wpool", bufs=1) as wpool,
            tc.tile_pool(name="xpool", bufs=4) as xpool,
            tc.tile_pool(name="apool", bufs=3) as apool,
            tc.tile_pool(name="ypool", bufs=10) as ypool,
            tc.tile_pool(name="psum_g", bufs=1, space="PSUM") as psum_g,
            tc.tile_pool(name="psum_u", bufs=1, space="PSUM") as psum_u,
            tc.tile_pool(name="psum_y", bufs=6, space="PSUM") as psum_y,
        ):
            wg_s = wpool.tile([128, KT, I], dt, tag="wg")
            wu_s = wpool.tile([128, KT, I], dt, tag="wu")
            wd_s = wpool.tile([128, H], dt, tag="wd")
            cf_s = wpool.tile([128, C // 128], f32, tag="cf")
            # Startup order on the sync ring: wg, then the FIRST x tile, then
            # wu/wd/cf — the first gate matmul only needs wg + one x quarter,
            # and wu/wd are not needed until the up/down phases ~4-10us later.
            tiles = _token_tiles(C)
            xt0_s = xpool.tile([128, KT, 512], dt, tag="x", name="xt0_s")[:, :, :tiles[0]]
            # interleave wg quarters (128KB, contiguous 1KB/partition runs in
            # the partition-major layout) with the first x tile's quarters so
            # the k=0 gate matmul starts after ~0.4MB of DMA; the x quarters
            # alternate across BOTH rings (both idle at startup) to halve the
            # ramp that paces tile 0's gate matmuls
            for qi, q0 in enumerate(range(0, KT, 4)):
                nc.sync.dma_start(wg_s[:, q0:q0 + 4, :], wg[:, q0:q0 + 4, :])
                leng = nc.sync if qi % 2 == 0 else nc.scalar
                leng.dma_start(
                    xt0_s[:, q0:q0 + 4, :],
                    xt[q0:q0 + 4, :, 0:tiles[0]].rearrange("k p t -> p k t"))
            nc.sync.dma_start(wu_s[:], wu[:])
            nc.sync.dma_start(wd_s[:], wd[:])
            nc.sync.dma_start(cf_s[:], cf[:])

            def emit_down(a_s, TT, t0):
                # down projection per 128-token subtile; scale by c[t] on the
                # PSUM->SBUF copy (ACT/DVE alternate per 512-col chunk);
                # store rows to HBM
                for j in range(TT // 128):
                    jj = t0 // 128 + j
                    y_s = ypool.tile([128, H], odt, tag="ys", name="y_s")
                    for ci, n0 in enumerate(range(0, H, 512)):
                        y_ps = psum_y.tile([128, 512], f32, tag="yp", name="y_ps")
                        nc.tensor.matmul(y_ps, lhsT=a_s[:, j * 128:(j + 1) * 128],
                                         rhs=wd_s[:, n0:n0 + 512],
                                         start=True, stop=True)
                        if ci % 2 == 0:
                            nc.scalar.activation(y_s[:, n0:n0 + 512], y_ps,
                                                 AF.Copy, scale=cf_s[:, jj:jj + 1])
                        else:
                            nc.vector.tensor_scalar_mul(y_s[:, n0:n0 + 512], y_ps,
                                                        cf_s[:, jj:jj + 1])
                    # stores alternate between the two HWDGE rings
                    eng = nc.scalar if jj % 2 == 0 else nc.sync
                    eng.dma_start(y[t0 + j * 128: t0 + (j + 1) * 128, :], y_s[:])

            t0 = 0
            prev = None
            for ti, TT in enumerate(tiles):
                if ti == 0:
                    xt_s = xt0_s
                else:
                    xt_s = xpool.tile([128, KT, 512], dt, tag="x",
                                      name="xt_s")[:, :, :TT]
                    # quarter-granularity loads so gate matmuls start after
                    # the first 4 k-slices land instead of the whole tile
                    for q0 in range(0, KT, 4):
                        nc.sync.dma_start(
                            xt_s[:, q0:q0 + 4, :],
                            xt[q0:q0 + 4, :, t0:t0 + TT].rearrange("k p t -> p k t"))

                # gate / up projections: gT/uT [I, TT] accumulated over KT
                g_ps = psum_g.tile([128, 512], f32, tag="g", name="g_ps")[:, :TT]
                u_ps = psum_u.tile([128, 512], f32, tag="u", name="u_ps")[:, :TT]
                for k in range(KT):
                    nc.tensor.matmul(g_ps, lhsT=wg_s[:, k, :], rhs=xt_s[:, k, :],
                                     start=(k == 0), stop=(k == KT - 1))
                for k in range(KT):
                    nc.tensor.matmul(u_ps, lhsT=wu_s[:, k, :], rhs=xt_s[:, k, :],
                                     start=(k == 0), stop=(k == KT - 1))

                # a = silu(g) * u   [I, TT] in compute dtype
                sg_s = apool.tile([128, 512], f32, tag="sg", name="sg_s")[:, :TT]
                a_s = apool.tile([128, 512], dt, tag="a", name="a_s")[:, :TT]
                nc.scalar.activation(sg_s, g_ps, AF.Silu)
                nc.vector.tensor_mul(a_s, sg_s, u_ps)

                # software pipeline: emit the PREVIOUS tile's down phase now,
                # so this tile's gate/up matmuls run on the PE while the
                # previous tile's PSUM->SBUF copies drain
                if prev is not None:
                    emit_down(*prev)
                prev = (a_s, TT, t0)
                t0 += TT
            emit_down(*prev)

    nc.finalize()
    _BUILD_CACHE[key] = nc
    return nc


def kernel(hidden_states, gate_w, w_gate, w_up, w_down):
    global LAST_RESULTS
    hidden_states = np.asarray(hidden_states, dtype=np.float32)
    gate_w = np.asarray(gate_w, dtype=np.float32)
    w_gate = np.asarray(w_gate, dtype=np.float32)
    w_up = np.asarray(w_up, dtype=np.float32)
    w_down = np.asarray(w_down, dtype=np.float32)

    B, S, _ = hidden_states.shape
    T = B * S
    x = np.ascontiguousarray(hidden_states.reshape(T, H))

    # --- host router: logits -> softmax -> top-1 (this is the sharding step) ---
    logits = x @ gate_w.T                                   # [T, E] fp32
    m = logits.max(axis=1, keepdims=True)
    ex = np.exp(logits - m)
    probs = ex / ex.sum(axis=1, keepdims=True)              # [T, E] fp32
    sel = np.argmax(logits, axis=1)                         # ties -> first, as jnp
    cvals = probs[np.arange(T), sel].astype(np.float32)     # top-1 prob per token

    idx = [np.nonzero(sel == e)[0] for e in range(E)]
    maxc = max(len(i) for i in idx)
    C = max(128, ((maxc + 127) // 128) * 128)

    dt, npdt = _DT[COMPUTE_DT]
    in_maps = []
    for e in range(E):
        ie = idx[e]
        pad = C - len(ie)
        ie_p = np.concatenate([ie, np.zeros(pad, dtype=ie.dtype)]) if pad else ie
        c_p = cvals[ie_p].copy()
        if pad:
            c_p[len(ie):] = 0.0
        xte = np.ascontiguousarray(x[ie_p].T).reshape(KT, 128, C)
        in_maps.append({
            "xt": xte.astype(npdt, copy=False) if npdt is np.float32 else xte.astype(npdt),
            "wg": np.ascontiguousarray(
                w_gate[e].T.reshape(KT, 128, I).transpose(1, 0, 2)).astype(npdt),
            "wu": np.ascontiguousarray(
                w_up[e].T.reshape(KT, 128, I).transpose(1, 0, 2)).astype(npdt),
            "wd": np.ascontiguousarray(w_down[e].T).astype(npdt),
            "cf": np.ascontiguousarray(c_p.reshape(C // 128, 128).T),
        })

    nc = _build(C, COMPUTE_DT, OUT_DT)
    res = run_bass_kernel_spmd(nc, in_maps, core_ids=list(range(NCORES)))
    LAST_RESULTS = res

    final = np.empty((T, H), dtype=np.float32)
    for e in range(E):
        ie = idx[e]
        if len(ie):
            final[ie] = res.results[e]["y"][:len(ie)].astype(np.float32, copy=False)

    hn = np.zeros((1, B, E * 4), dtype=np.float32)
    return (
        final.reshape(B, S, H),
        probs,
        hn,
        np.float32(0.035),
        _COS_SIM.copy(),
        np.float32(0.019),
    )
